# revision 7
# baseline (speedup 1.0000x reference)
"""Trainium2 Bass kernel for the SNN leaky-integrate-and-fire problem.

Reference semantics (per batch row b, channels h=224, time t=224):
    x = roll(inp, 57, axis=time)
    T(b,t) = 3 + 2*tanh(dot(x[b,:,t], w))        (clip(1,5) is a no-op: 3+2*tanh is in [1,5])
    mem(t) = beta*mem(t-1) + x(t) - T(t)*[mem(t-1) > T(t)]
    spk(t) = [mem(t) > T(t)]
    out[b, 0, h, t] = spk

Sharding: pure data parallelism over batch (1024 -> 8 cores x 128). The
128-row batch shard maps exactly onto the 128 SBUF partitions; h rides the
free dimension, and the t recurrence runs as a sequence of [128, 224] vector
ops. w is replicated.
"""

import os
from contextlib import ExitStack

import numpy as np

import concourse.bass as bass
import concourse.tile as tile
from concourse import bacc, bass_utils, mybir

F32 = mybir.dt.float32
Alu = mybir.AluOpType
Act = mybir.ActivationFunctionType

CH = 224           # channels (h)
TT = 224           # time steps
ROLL = 57
BETA = 0.95
N_CORES = 8
BATCH = 1024
BPC = BATCH // N_CORES   # 128 = SBUF partitions


def _blocks(total, size):
    """[(t0, len)] covering range(total) in chunks of `size`."""
    out = []
    t0 = 0
    while t0 < total:
        out.append((t0, min(size, total - t0)))
        t0 += size
    return out


def _rolled_segments(t0, ln, total, roll):
    """DRAM time segments for rolled block [t0, t0+ln): x_rolled[t] = inp[(t - roll) % total].

    Returns [(dst_off, src_t0, seg_len)]."""
    src0 = (t0 - roll) % total
    if src0 + ln <= total:
        return [(0, src0, ln)]
    first = total - src0
    return [(0, src0, first), (first, 0, ln - first)]


def lif_kernel(ctx, tc, out, inp, w, b=BPC, ch=CH, tt=TT, roll=ROLL,
               tc_block=32, spk_engine="vector"):
    """Emit the LIF kernel body. out/inp/w are DRAM APs."""
    nc = tc.nc
    if True:
        pers = ctx.enter_context(tc.tile_pool(name="pers", bufs=1))
        xpool = ctx.enter_context(tc.tile_pool(name="x", bufs=2))
        spool = ctx.enter_context(tc.tile_pool(name="spk", bufs=2))
        tpool = ctx.enter_context(tc.tile_pool(name="thr", bufs=2))
        psum = ctx.enter_context(tc.tile_pool(name="ps", bufs=1, space="PSUM"))

        # ---- persistent state ----
        mem = pers.tile([b, ch], F32, tag="mem")    # membrane potential
        u = pers.tile([b, ch], F32, tag="u")        # beta*mem + x scratch
        rT = pers.tile([b, ch], F32, tag="rT")      # reset*T scratch
        junk = pers.tile([b, ch], F32, tag="junk")  # STT elementwise product sink
        wb = pers.tile([b, ch], F32, tag="wb")      # w broadcast to all partitions
        w_sb = pers.tile([1, ch], F32, tag="wsb")
        ones = pers.tile([1, b], F32, tag="ones")

        nc.vector.memset(mem[:], 0.0)
        nc.gpsimd.memset(ones[:], 1.0)
        nc.sync.dma_start(w_sb[:], w[None, :])

        # broadcast w to 128 partitions via a K=1 outer product on the PE
        wb_ps = psum.tile([b, ch], F32, tag="wbps")
        nc.tensor.matmul(wb_ps[:], ones[:], w_sb[:], start=True, stop=True)
        nc.scalar.copy(wb[:], wb_ps[:])

        spk_eng = getattr(nc, spk_engine)

        for t0, ln in _blocks(tt, tc_block):
            # ---- load x block (rolled time order), layout [b, ch, ln] ----
            xb = xpool.tile([b, ch, ln], F32, tag="x")
            for dst, src_t0, seg in _rolled_segments(t0, ln, tt, roll):
                nc.sync.dma_start(
                    xb[:, :, dst:dst + seg], inp[:, :, src_t0:src_t0 + seg]
                )

            # ---- thresholds for the block: T = 3 + 2*tanh(x_t . w) ----
            dots = tpool.tile([b, ln], F32, tag="dots")
            for tl in range(ln):
                nc.vector.scalar_tensor_tensor(
                    junk[:], xb[:, :, tl], 1.0, wb[:],
                    op0=Alu.mult, op1=Alu.mult,
                    accum_out=dots[:, tl:tl + 1],
                )
            tanh = tpool.tile([b, ln], F32, tag="tanh")
            nc.scalar.activation(tanh[:], dots[:], Act.Tanh)
            thr = tpool.tile([b, ln], F32, tag="thr")
            nc.vector.tensor_scalar(thr[:], tanh[:], 2.0, 3.0, op0=Alu.mult, op1=Alu.add)

            # ---- recurrence over the block ----
            spk = spool.tile([b, ch, ln], F32, tag="spk")
            for tl in range(ln):
                tcol = thr[:, tl:tl + 1]
                # rT = T * (mem > T)   (uses mem from step t-1)
                nc.vector.tensor_scalar(
                    rT[:], mem[:], tcol, tcol, op0=Alu.is_gt, op1=Alu.mult
                )
                # u = beta*mem + x_t   (matches reference association order)
                nc.vector.scalar_tensor_tensor(
                    u[:], mem[:], BETA, xb[:, :, tl], op0=Alu.mult, op1=Alu.add
                )
                # mem = u - rT
                nc.vector.tensor_sub(mem[:], u[:], rT[:])
                # spk_t = (mem > T)
                spk_eng.tensor_scalar(
                    spk[:, :, tl], mem[:], tcol, None, op0=Alu.is_gt
                )

            # ---- store spikes ----
            nc.sync.dma_start(out[:, 0, :, t0:t0 + ln], spk[:, :, :])


def build_kernel(b=BPC, ch=CH, tt=TT, roll=ROLL, tc_block=32, spk_engine="vector"):
    """Build the per-core Bass program. Returns the compiled Bacc object."""
    nc = bacc.Bacc()
    inp = nc.dram_tensor("inp", [b, ch, tt], F32, kind="ExternalInput")
    w = nc.dram_tensor("w", [ch], F32, kind="ExternalInput")
    out = nc.dram_tensor("out", [b, 1, ch, tt], F32, kind="ExternalOutput")

    with tile.TileContext(nc) as tc:
        with ExitStack() as ctx:
            lif_kernel(ctx, tc, out, inp, w, b=b, ch=ch, tt=tt, roll=roll,
                       tc_block=tc_block, spk_engine=spk_engine)

    nc.compile()
    return nc


_NC_CACHE = {}


def _get_nc():
    key = "default"
    if key not in _NC_CACHE:
        _NC_CACHE[key] = build_kernel()
    return _NC_CACHE[key]


def kernel(inp: np.ndarray, w: np.ndarray) -> np.ndarray:
    inp = np.ascontiguousarray(inp, dtype=np.float32)
    w = np.ascontiguousarray(w, dtype=np.float32)
    assert inp.shape == (BATCH, CH, TT) and w.shape == (CH,)

    nc = _get_nc()
    shards = np.split(inp, N_CORES, axis=0)
    in_maps = [{"inp": s, "w": w} for s in shards]
    trace = bool(int(os.environ.get("LIF_TRACE", "0")))
    res = bass_utils.run_bass_kernel_spmd(
        nc, in_maps, core_ids=list(range(N_CORES)), trace=trace
    )
    kernel.last_results = res
    outs = [r["out"] for r in res.results]
    return np.concatenate(outs, axis=0)
